# revision 54
# baseline (speedup 1.0000x reference)
"""Multi-head causal attention (B=4, S=2048, D=1024, H=16) on 8 NeuronCores.

Sharding: core c handles batch b = c//2 and head-group g = c%2 (8 heads).
Each core computes QKV projections for its group, causal attention for its
8 heads, and a partial output projection (row-split Wo).  Host sums the two
f32 partials per batch and adds bo.

Numerics / engine plan (per core):
  - Q/K projections: raw fp8(E4M3) DoubleRow matmuls (K=256 per pass) on
    Xhi and 32*W quantized host-side; evac adds bias and rescales to
    8*Q / 8*K directly in fp8 via a fused DVE tensor_scalar.
  - V projection: 3-term compensated fp8-DR -- X = Xhi + Xlo and
    Wv = Wvhi + Wvlo split host-side; psum = Xhi@Whi + Xlo@Whi + Xhi@Wlo
    (the Xlo@Wlo term is negligible).  Evac adds 32*bv and keeps V scaled
    by 32 in fp16; the 1/32 rides through PV and is removed in the output
    projection evac.
  - Scores: fp8 DoubleRow with head-dim folded [32,2] (K=64); causal diag
    masking via an fp16 identity x (-30000) matmul accumulated into the
    same psum group; exp on ACT with scale 1/512 (folds the 1/sqrt(64)
    and the two 8x fp8 scale factors), fp16 out.
  - PV: fp16, re-oriented for full PE utilization: out[q=128, 65] with
    lhsT = exp-scores chunk [k=128, q=128], rhs = V[k=128, 64+indicator];
    col 64 gives the softmax row-sum.  Both heads accumulate start=False
    into one 2KB psum bank pre-zeroed by a K=1 ones x zeros matmul (PE
    in-order, no extra sems; GPSIMD cannot write PSUM on hardware).
  - Normalize: DVE reciprocal of the rowsum column + per-partition
    tensor_scalar multiply (q is on partitions, so no broadcast needed).
  - O^T via PE f32 transpose (start=True, upper region of the pv bank),
    output projection in fp16 against host-prescaled Wo/32, f32 DMA out.

PSUM budget (8 banks): proj 2x[128,512]f32 + scores 2x[128,2,512]f32 +
pv/transpose 2x[128,512]f32 = exactly 8.

Emission uses a deferred-work queue: per-slice pv/normalize/transpose
blocks and next-pair QK-projection groups are interleaved into the next
slice's score/exp t-loop so the ACT engine (the pacing engine) never
waits on PE program order.

Walrus wait-slot legality (1 sem wait per engine instruction) is restored
by splitting extra waits onto same-engine NoOps.
"""

import sys

for _p in ("/opt/trn_rl_repo",):
    if _p not in sys.path:
        sys.path.insert(0, _p)

from contextlib import ExitStack

import numpy as np
import ml_dtypes

import concourse.bass as bass
import concourse.mybir as mybir
import concourse.tile as tile
from concourse.bass_utils import run_bass_kernel_spmd

import bass_rust

E4np = ml_dtypes.float8_e4m3
F8 = mybir.dt.float8e4
F16 = mybir.dt.float16
F32 = mybir.dt.float32
AF = mybir.ActivationFunctionType
ALU = mybir.AluOpType
DR = mybir.MatmulPerfMode.DoubleRow

B, S, D, H = 4, 2048, 1024, 16
HD = D // H  # 64
GH = 8  # heads per group
GW = GH * HD  # 512 columns per group
MASKVAL = -30000.0
LAM = 0.125 / 64.0  # exp scale: 1/sqrt(64) / (8*8)


_SPLITTABLE = {
    "InstMatmult", "InstLdweights", "InstActivation", "InstTensorCopy",
    "InstTensorTensor", "InstTensorScalarPtr", "InstTensorReduce",
    "InstMemset", "InstDMACopy", "InstReciprocal", "InstIota",
    "InstTensorTensorReduce", "InstBNStats", "InstBNStatsAggregate",
    "InstStreamShuffle", "InstNoOp", "InstPool", "InstMax", "InstDrain",
}


def _legalize_waits(nc, max_waits=1):
    """Walrus codegen accepts at most one sync-wait command per engine
    instruction; Tile's wait assigner can emit more.  Split extras onto
    same-engine NoOps inserted immediately before (semantics preserved:
    the engine blocks at the same program point)."""
    ctr = 0
    for fn in nc.m.functions:
        for blk in fn.blocks:
            out = []
            for ins in blk.instructions:
                si = ins.sync_info
                if (
                    si is not None
                    and len(si.on_wait) > max_waits
                    and type(ins).__name__ in _SPLITTABLE
                ):
                    waits = list(si.on_wait)
                    extra, keep = waits[:-max_waits], waits[-max_waits:]
                    for w in extra:
                        nop = mybir.InstNoOp(name=f"waitnop-{ctr}", ins=[], outs=[])
                        ctr += 1
                        nop.engine = ins.engine
                        nop.sync_info = bass_rust.SyncInfo(on_wait=[w], on_update=[])
                        out.append(nop)
                    ins.sync_info = bass_rust.SyncInfo(
                        on_wait=keep, on_update=list(si.on_update)
                    )
                out.append(ins)
            blk.instructions[:] = out
    return ctr


def build_nc(s=S, legalize=True):
    nt = s // 128  # 128-wide s chunks
    ns = s // 512  # 512-wide q slices per head
    nd = D // 128  # contraction chunks for projections

    nc = bass.Bass("TRN2", target_bir_lowering=False, debug=False)
    xhi_d = nc.dram_tensor("xhi", [D, s], F8, kind="ExternalInput").ap()
    xlo_d = nc.dram_tensor("xlo", [D, s], F8, kind="ExternalInput").ap()
    wq_d = nc.dram_tensor("wq", [D, GW], F8, kind="ExternalInput").ap()
    wk_d = nc.dram_tensor("wk", [D, GW], F8, kind="ExternalInput").ap()
    wvh_d = nc.dram_tensor("wvh", [D, GW], F8, kind="ExternalInput").ap()
    wvl_d = nc.dram_tensor("wvl", [D, GW], F8, kind="ExternalInput").ap()
    wo_d = nc.dram_tensor("wo", [GW, D], F16, kind="ExternalInput").ap()
    bqk_d = nc.dram_tensor("bqk", [128, 8], F32, kind="ExternalInput").ap()
    bvb_d = nc.dram_tensor("bvb", [128, GW], F16, kind="ExternalInput").ap()
    mask_d = nc.dram_tensor("mask", [128, 128], F16, kind="ExternalInput").ap()
    ident_d = nc.dram_tensor("ident", [128, 128], F32, kind="ExternalInput").ap()
    ident16_d = nc.dram_tensor("ident16", [128, 128], F16, kind="ExternalInput").ap()
    out_d = nc.dram_tensor("out", [s, D], F32, kind="ExternalOutput").ap()

    with tile.TileContext(nc) as tc, ExitStack() as ctx:
        pool = lambda name, bufs, **kw: ctx.enter_context(
            tc.tile_pool(name=name, bufs=bufs, **kw)
        )
        const_p = pool("const", 1)
        xt_p = pool("xtp", 1)
        w_p = pool("wp", 1)
        qtmp_p = pool("qtmpp", 2)
        qk8_p = pool("qk8p", 2)
        et_p = pool("etp", 32)
        v_p = pool("vp", nt)
        on_p = pool("onp", 2)
        rs_p = pool("rsp", 2)
        ot_p = pool("otp", 1)
        ob_p = pool("obp", 4)
        ps_proj = pool("psproj", 2, space="PSUM")
        ps_qk = pool("psqk", 2, space="PSUM")
        ps_pv = pool("pspv", 2, space="PSUM")

        # ---- input DMAs, in criticality order (SP issues in program
        # order; transfers serialize on the DMA engines, so order = need)
        xhi_sb = xt_p.tile([128, nd, s], F8)
        xlo_sb = xt_p.tile([128, nd, s], F8)
        wq_sb = w_p.tile([128, nd, GW], F8)
        wk_sb = w_p.tile([128, nd, GW], F8)
        wvh_sb = w_p.tile([128, nd, GW], F8)
        wvl_sb = w_p.tile([128, nd, GW], F8)

        def dma_x(dst, src, sl0, sl1):
            nc.sync.dma_start(
                out=dst[:, :, sl0 * 512 : sl1 * 512],
                in_=src.rearrange("(d p) n -> p d n", p=128)[
                    :, :, sl0 * 512 : sl1 * 512
                ],
            )

        def dma_w(dst, src, c):
            nc.sync.dma_start(
                out=dst[:, :, c * 128 : (c + 1) * 128],
                in_=src.rearrange("(d p) n -> p d n", p=128)[
                    :, :, c * 128 : (c + 1) * 128
                ],
            )

        # ACT exp-table preload off the critical path
        scr_in = const_p.tile([128, 1], F32)
        nc.gpsimd.memset(scr_in[:], 0.0)
        scr_out = const_p.tile([128, 1], F16)
        nc.scalar.activation(scr_out[:], scr_in[:], AF.Exp, scale=1.0)
        # constants for the PE-side psum zeroing matmul (GPSIMD cannot
        # write PSUM on real hardware)
        ones1 = const_p.tile([1, 128], F16)
        nc.gpsimd.memset(ones1[:], 1.0)
        zer1 = const_p.tile([1, 512], F16)
        nc.gpsimd.memset(zer1[:], 0.0)

        dma_x(xhi_sb, xhi_d, 0, 1)
        dma_w(wq_sb, wq_d, 0)
        dma_w(wk_sb, wk_d, 0)
        bqk_sb = const_p.tile([128, 8], F32)
        nc.sync.dma_start(out=bqk_sb[:], in_=bqk_d[:])
        ident16_sb = const_p.tile([128, 128], F16)
        nc.sync.dma_start(out=ident16_sb[:], in_=ident16_d[:])
        mask_sb = const_p.tile([128, 128], F16)
        nc.sync.dma_start(out=mask_sb[:], in_=mask_d[:])
        dma_x(xhi_sb, xhi_d, 1, 2)
        dma_w(wq_sb, wq_d, 1)
        dma_w(wk_sb, wk_d, 1)
        dma_x(xhi_sb, xhi_d, 2, ns)

        def dma_wv(half):
            hs = slice(half * nd // 2, (half + 1) * nd // 2)
            nc.sync.dma_start(
                out=wvh_sb[:, hs, :],
                in_=wvh_d.rearrange("(d p) n -> p d n", p=128)[:, hs, :],
            )
            nc.sync.dma_start(
                out=wvl_sb[:, hs, :],
                in_=wvl_d.rearrange("(d p) n -> p d n", p=128)[:, hs, :],
            )

        dma_wv(0)
        dma_x(xlo_sb, xlo_d, 0, 1)
        dma_wv(1)
        bvb_sb = const_p.tile([128, GW], F16)
        nc.sync.dma_start(out=bvb_sb[:], in_=bvb_d[:])
        ident_sb = const_p.tile([128, 128], F32)
        nc.sync.dma_start(out=ident_sb[:], in_=ident_d[:])
        dma_x(xlo_sb, xlo_d, 1, ns)
        for c in (2, 3):
            dma_w(wq_sb, wq_d, c)
            dma_w(wk_sb, wk_d, c)
        wo_sb = w_p.tile([128, 4, D], F16)
        nc.sync.dma_start(out=wo_sb[:], in_=wo_d.rearrange("(c p) n -> p c n", p=128))

        # persistent tiles
        v_sb = [v_p.tile([128, GH, 65], F16, tag="v", name=f"v{t}") for t in range(nt)]
        for t in range(nt):
            nc.gpsimd.memset(v_sb[t][:, :, 64:65], 1.0)
        otT = ot_p.tile([128, 4, s], F16)

        # ---- deferred-work queue (credit-paced) ----
        # Tile's monotonic per-engine progress semaphores make every exp
        # wait on ALL earlier-emitted PE work, so deferred PE bursts must
        # be paced against the ACT time the exps provide.
        deferred = []  # (pe_weight_us, fn, et_watermark)
        credit = [0.0]
        et_ctr = [0]
        ET_BUFS = 32

        def pop_one():
            w, fn, _ = deferred.pop(0)
            fn()
            credit[0] -= w

        def pop_deferred(exp_us=0.0, sc_us=0.0):
            credit[0] = min(credit[0] + 0.85 * exp_us - sc_us, 2.5)
            while deferred and credit[0] >= deferred[0][0]:
                pop_one()

        def et_watermark_drain():
            # a qi-block must be emitted before the et slots it reads get
            # reassigned (Tile tracks WARs against readers known at
            # emission time); called before every et alloc
            while deferred and deferred[0][2] <= et_ctr[0] - (ET_BUFS - 1):
                pop_one()

        def push(w, fn, wm=None):
            deferred.append((w, fn, et_ctr[0] if wm is None else wm))

        # ---- projection helpers ----
        def emit_qk_slice(c, sl, qki, tmp, pk):
            """One Q-or-K projection group for (pair c, slice sl) plus the
            fused bias+rescale fp8 evac; after the K group, the four pack
            DMAs that fold hd 32:64 onto partitions 0:32 (slot 1)."""
            wsb, bcol = ((wq_sb, c), (wk_sb, 4 + c))[qki]
            ps = ps_proj.tile([128, 512], F32, tag="ps", name="ps")
            for dp in range(nd // 2):
                nc.tensor.matmul(
                    ps[:],
                    wsb[:, 2 * dp : 2 * dp + 2, c * 128 : (c + 1) * 128],
                    xhi_sb[:, 2 * dp : 2 * dp + 2, sl * 512 : (sl + 1) * 512],
                    start=(dp == 0),
                    stop=(dp == nd // 2 - 1),
                    perf_mode=DR,
                )
            # (psum + 32b) * 0.25 = 8 * (psum/32 + b) -> fp8
            nc.vector.tensor_scalar(
                tmp[:, qki, sl * 512 : (sl + 1) * 512],
                ps[:],
                bqk_sb[:, bcol : bcol + 1],
                0.25,
                ALU.add,
                ALU.mult,
            )
            if qki == 1:
                # pair 0's first slice gates the whole pipeline: issue its
                # packs from the still-idle ACT (fast HWDGE) instead of
                # Pool (slow SWDGE); later pair-0 slices split Pool/SP to
                # halve the serial SWDGE-generation latency
                eng = nc.scalar if (c == 0 and sl == 0) else nc.gpsimd
                for g in range(4):
                    eng.dma_start(
                        out=pk[
                            (g // 2) * 32 : (g // 2) * 32 + 32,
                            g % 2,
                            :,
                            sl * 512 : (sl + 1) * 512,
                        ],
                        in_=tmp[g * 32 : (g + 1) * 32, :, sl * 512 : (sl + 1) * 512],
                    )

        def emit_qk_proj(c):
            tmp = qtmp_p.tile([128, 2, s], F8, tag="qtmp", name=f"qtmp{c}")
            pk = qk8_p.tile([64, 2, 2, s], F8, tag="qk8", name=f"pk{c}")
            for sl in range(ns):
                for qki in range(2):
                    emit_qk_slice(c, sl, qki, tmp, pk)
            return pk

        def qk_proj_items(c, acc):
            """Deferred per-(slice, q/k) projection items; tiles allocated
            lazily by the first item, the pk handle lands in `acc`."""
            state = {}

            def first():
                state["tmp"] = qtmp_p.tile([128, 2, s], F8, tag="qtmp", name=f"qtmp{c}")
                pk = qk8_p.tile([64, 2, 2, s], F8, tag="qk8", name=f"pk{c}")
                state["pk"] = pk
                acc.append(pk)

            items = []
            for sl in range(ns):
                for qki in range(2):
                    def item(sl=sl, qki=qki, isfirst=(sl == 0 and qki == 0)):
                        if isfirst:
                            first()
                        emit_qk_slice(c, sl, qki, state["tmp"], state["pk"])

                    items.append((0.55, item))
            return items

        V_TERMS = [(xhi_sb, wvh_sb), (xlo_sb, wvh_sb), (xhi_sb, wvl_sb)]

        def _v_half(st, half, state):
            if half == 0:
                state["ps"] = ps_proj.tile([128, 512], F32, tag="ps", name="ps")
            ps = state["ps"]
            n = len(V_TERMS) * nd // 2
            idx = [(ti, dp) for ti in range(len(V_TERMS)) for dp in range(nd // 2)]
            for i in range(half * n // 2, (half + 1) * n // 2):
                ti, dp = idx[i]
                xsb, wsb = V_TERMS[ti]
                nc.tensor.matmul(
                    ps[:],
                    xsb[:, 2 * dp : 2 * dp + 2, st * 128 : (st + 1) * 128],
                    wsb[:, 2 * dp : 2 * dp + 2, :],
                    start=(i == 0),
                    stop=(i == n - 1),
                    perf_mode=DR,
                )
            if half == 1:
                nc.vector.tensor_add(
                    v_sb[st][:, :, 0:64],
                    ps[:].rearrange("p (h e) -> p h e", h=GH),
                    bvb_sb[:].rearrange("p (h e) -> p h e", h=GH),
                )

        def v_proj_items(st):
            state = {}
            return [
                (0.72, lambda st=st, state=state: _v_half(st, 0, state)),
                (0.72, lambda st=st, state=state: _v_half(st, 1, state)),
            ]

        # ---- O-projection item (deferred; c==3 only, per q-chunk) ----
        def make_oproj_item(qi, act_evac=False):
            def emit():
                for dsl in range(2):
                    po = ps_proj.tile([128, 512], F32, tag="ps", name="ps")
                    for cc in range(4):
                        nc.tensor.matmul(
                            po[:],
                            otT[:, cc, qi * 128 : (qi + 1) * 128],
                            wo_sb[:, cc, dsl * 512 : (dsl + 1) * 512],
                            start=(cc == 0),
                            stop=(cc == 3),
                        )
                    ob = ob_p.tile([128, 512], F32, tag="ob", name="ob")
                    if act_evac:
                        nc.scalar.activation(ob[:], po[:], AF.Copy, scale=1.0)
                    else:
                        nc.vector.tensor_copy(ob[:], po[:])
                    nc.sync.dma_start(
                        out=out_d[
                            qi * 128 : (qi + 1) * 128, dsl * 512 : (dsl + 1) * 512
                        ],
                        in_=ob[:],
                    )

            return (1.75, emit)

        # ---- attention qi-block (deferred) ----
        def make_qi_block(c, qi, et_tiles, act_evac=False):
            weight = (qi + 1) * 0.054 + 0.11

            def emit():
                pv = ps_pv.tile([128, 512], F32, tag="pv", name="pv")
                pv3 = pv[:, 0:130].rearrange("p (a b) -> p a b", a=2)
                # zero the accumulation region with a K=1 matmul: same
                # engine as the accumulating matmuls (in-order, no sems)
                # and its start=True resets the bank's zero-region state
                nc.tensor.matmul(
                    pv[:, 0:130],
                    ones1[:],
                    zer1[0:1, 0:130],
                    start=True,
                    stop=True,
                    skip_group_check=True,
                )
                for t2 in range(qi + 1):
                    for hh in range(2):
                        nc.tensor.matmul(
                            pv3[:, hh, 0:65],
                            et_tiles[t2][:, hh, (qi % 4) * 128 : (qi % 4 + 1) * 128],
                            v_sb[t2][:, 2 * c + hh, 0:65],
                            start=False,
                            stop=(t2 == qi),
                            skip_group_check=True,
                        )
                rs = rs_p.tile([128, 2], F32, tag="rs", name="rs")
                nc.vector.reciprocal(rs[:], pv3[:, :, 64:65])
                on = on_p.tile([128, 2, 64], F32, tag="on", name="on")
                for hh in range(2):
                    nc.vector.tensor_scalar_mul(
                        on[:, hh, :], pv3[:, hh, 0:64], rs[:, hh : hh + 1]
                    )
                nc.tensor.matmul(
                    pv[:, 384:512],
                    on[:],
                    ident_sb[:],
                    start=True,
                    stop=True,
                    is_transpose=True,
                    skip_group_check=True,
                )
                if act_evac:
                    nc.scalar.activation(
                        otT[:, c, qi * 128 : (qi + 1) * 128],
                        pv[:, 384:512],
                        AF.Copy,
                        scale=1.0,
                    )
                else:
                    nc.vector.tensor_copy(
                        otT[:, c, qi * 128 : (qi + 1) * 128], pv[:, 384:512]
                    )

            return (weight, emit)

        # ---- main emission ----
        pk = emit_qk_proj(0)

        for c in range(4):
            qi_stash = {}
            for j in range(ns):
                slice_et0 = et_ctr[0]
                et_tiles = []
                for t in range(4 * j + 4):
                    diag = t >= 4 * j
                    w0 = 128 * (t - 4 * j) if diag else 0
                    qk = ps_qk.tile([128, 2, 512], F32, tag="qk", name="qk")
                    for hh in range(2):
                        nc.tensor.matmul(
                            qk[:, hh, w0:512],
                            pk[hh * 32 : hh * 32 + 32, :, 1, t * 128 : (t + 1) * 128],
                            pk[
                                hh * 32 : hh * 32 + 32,
                                :,
                                0,
                                j * 512 + w0 : (j + 1) * 512,
                            ],
                            start=True,
                            stop=not diag,
                            perf_mode=DR,
                            skip_group_check=True,
                        )
                        if diag:
                            nc.tensor.matmul(
                                qk[:, hh, w0 : w0 + 128],
                                ident16_sb[:],
                                mask_sb[:],
                                start=False,
                                stop=True,
                                skip_group_check=True,
                            )
                    et_watermark_drain()
                    et = et_p.tile([128, 2, 512], F16, tag="et", name="et")
                    et_ctr[0] += 1
                    nc.scalar.activation(
                        et[:, :, w0:512], qk[:, :, w0:512], AF.Exp, scale=LAM
                    )
                    et_tiles.append(et)
                    ww = 512 - w0
                    pop_deferred(
                        exp_us=(2 * ww * 0.833 + 185) / 1000,
                        sc_us=(2 * ww * 0.5 * 0.417) / 1000 + (0.11 if diag else 0),
                    )
                    if c >= 1 and j == ns - 1 and t >= 12:
                        # tail qi-blocks inline: exp(t) just emitted and
                        # t == qi, so all inputs exist (pairs 1..3; pair 0
                        # is already oversubscribed).  For pair 3 the
                        # O-projections trail one t so they overlap the
                        # next exp; only the strictly post-last-exp work
                        # may use the in-order ACT engine for evacs.
                        make_qi_block(c, t, et_tiles, act_evac=(c == 3 and t == 15))[1]()
                        if c == 3 and t > 12:
                            make_oproj_item(t - 1)[1]()
                if c == 3 and j == ns - 1:
                    make_oproj_item(15, act_evac=True)[1]()
                qi_items = (
                    []
                    if (c >= 1 and j == ns - 1)
                    else [
                        make_qi_block(c, 4 * j + qi_l, et_tiles)
                        for qi_l in range(4)
                    ]
                )
                if c == 3 and j < ns - 1:
                    for qi_l in range(4):
                        qi_items.insert(
                            2 * qi_l + 1, make_oproj_item(4 * j + qi_l)
                        )
                if c == 0:
                    # pair 0 carries the V-projection; stagger pushes so
                    # the deferred queue drains roughly with the exps
                    qi_stash[j] = (slice_et0, qi_items)
                    if j == 0:
                        nxt = []
                        for w, fn in qk_proj_items(1, nxt):
                            push(w, fn)
                        next_qk = nxt
                    elif j == 1:
                        for st in range(4):
                            for w, fn in v_proj_items(st):
                                push(w, fn)
                        for wm, items in (qi_stash[0],):
                            for w, fn in items:
                                push(w, fn, wm)
                        for st in range(4, 8):
                            for w, fn in v_proj_items(st):
                                push(w, fn)
                    elif j == 2:
                        for st in range(8, 12):
                            for w, fn in v_proj_items(st):
                                push(w, fn)
                        for wm, items in (qi_stash[1], qi_stash[2]):
                            for w, fn in items:
                                push(w, fn, wm)
                    else:
                        # V(12..15) drain into pair 1's slack; they only
                        # gate qi(c0, j3) which pops during pair 1 anyway
                        for st in range(12, 16):
                            for w, fn in v_proj_items(st):
                                push(w, fn)
                        for wm, items in (qi_stash[3],):
                            for w, fn in items:
                                push(w, fn, wm)
                else:
                    for w, fn in qi_items:
                        push(w, fn, slice_et0)
                    if j == 0 and c < 3:
                        nxt = []
                        for w, fn in qk_proj_items(c + 1, nxt):
                            push(w, fn)
                        next_qk = nxt
            if c < 3:
                # switch to the next pair's packed q/k (created by the
                # deferred proj items, guaranteed popped during this pair)
                while not next_qk:
                    assert deferred, "deferred queue drained without proj item"
                    pop_one()
                pk = next_qk[0]

        while deferred:
            pop_one()

    if legalize:
        _legalize_waits(nc)
    return nc


_NC_CACHE = {}


def _get_nc(s=S):
    if s not in _NC_CACHE:
        _NC_CACHE[s] = build_nc(s)
    return _NC_CACHE[s]


def _q8(x, scale=1.0):
    return np.asarray(np.asarray(x, np.float32) * scale, E4np)


def make_inputs(X, Wq, bq, Wk, bk, Wv, bv, Wo, bo, s=S):
    """Per-core input maps. Core c: batch c//2, head group c%2."""
    iv, jv = np.arange(128)[:, None], np.arange(128)[None, :]
    maskneg = np.where(jv >= iv, 0.0, MASKVAL).astype(np.float16)
    ident = np.eye(128, dtype=np.float32)
    in_maps = []
    for c in range(8):
        b, g = divmod(c, 2)
        lo, hi = g * GW, (g + 1) * GW
        xt = np.ascontiguousarray(X[b, :s].T).astype(np.float32)
        xhi = _q8(xt)
        xlo = _q8(xt - xhi.astype(np.float32))
        wvt = np.ascontiguousarray(Wv[lo:hi].T).astype(np.float32)
        wvh = _q8(wvt, 32.0)
        wvl = _q8(wvt * 32.0 - wvh.astype(np.float32), 1.0)
        bqk = 32.0 * np.concatenate(
            [
                np.ascontiguousarray(bq[lo:hi].reshape(4, 128).T),
                np.ascontiguousarray(bk[lo:hi].reshape(4, 128).T),
            ],
            axis=1,
        ).astype(np.float32)
        in_maps.append(
            {
                "xhi": xhi,
                "xlo": xlo,
                "wq": _q8(np.ascontiguousarray(Wq[lo:hi].T), 32.0),
                "wk": _q8(np.ascontiguousarray(Wk[lo:hi].T), 32.0),
                "wvh": wvh,
                "wvl": wvl,
                "wo": (np.ascontiguousarray(Wo[:, lo:hi].T) / 32.0).astype(np.float16),
                "bqk": bqk.astype(np.float32),
                "bvb": np.tile(32.0 * bv[lo:hi].astype(np.float32), (128, 1)).astype(
                    np.float16
                ),
                "mask": maskneg,
                "ident": ident,
                "ident16": ident.astype(np.float16),
            }
        )
    return in_maps


def kernel(X, Wq, bq, Wk, bk, Wv, bv, Wo, bo, **run_kwargs):
    args = [np.asarray(a, np.float32) for a in (X, Wq, bq, Wk, bk, Wv, bv, Wo, bo)]
    X, Wq, bq, Wk, bk, Wv, bv, Wo, bo = args
    nc = _get_nc(S)
    in_maps = make_inputs(X, Wq, bq, Wk, bk, Wv, bv, Wo, bo, S)
    res = run_bass_kernel_spmd(nc, in_maps, core_ids=list(range(8)), **run_kwargs)
    outs = [r["out"] for r in res.results]
    full = np.empty((B, S, D), np.float32)
    for b in range(B):
        full[b] = outs[2 * b] + outs[2 * b + 1] + bo
    kernel.last_results = res
    return full
